# revision 1
# baseline (speedup 1.0000x reference)
"""Trainium2 Bass kernel for nn_Attention_26182120636812 (GQA attention block).

Sharding: 8 cores = 2 (batch) x 4 (KV groups). Each core computes, for its
batch element b and kv-group g: the 4 query heads + 1 kv head of group g,
full causal attention over T=2048, and the partial output projection
y_part = o_g @ wo[g*512:(g+1)*512, :]. The host sums the 4 partials per batch.

Device-side layout choices (see comments inline):
 - host passes x[b]^T so projection matmuls need no on-device transpose of x
 - head_dim is permuted host-side to [evens|odds] so RoPE works on contiguous
   halves; the permutation cancels in q.k and is never visible in the output
 - rmsnorm weights are folded into the rope cos/sin tensors host-side
 - scores are computed transposed (tk x tq) so softmax (no max subtraction --
   logits are bounded by sqrt(HD)=11.3 after rmsnorm) needs no transposes:
   exp on ACT, denominators via ones-matmul, 1/denom broadcast via K=1 matmul
 - causal masking: block skipping + split diagonal chunks + one 128x128 mask
 - matmuls run as float32r (full PE rate at N>=256); set MM_F32R=False for
   exact fp32 (4x slower on PE)
"""
import sys

for _p in ("/opt/trn_rl_repo",):
    if _p not in sys.path:
        sys.path.insert(0, _p)

import numpy as np

B, T, D = 2, 2048, 2048
H, KV, HD = 16, 4, 128
NCORES = 8
NH = H // KV          # 4 q heads per core
GW = NH * HD          # 512: per-core q / o width
TT = T // 128         # 16 token tiles
NI = T // 512         # 4 query super-tiles
DC = D // 128         # 16 contraction chunks over D
EPS = 1e-6
SCALE = 1.0 / float(np.sqrt(HD))
NEG = -1.0e30
MM_DTYPE = "f32r"      # "f32" | "f32r" | "bf16"

# diagonal chunk r in 0..3 of a 512-wide query tile: computed tq offset/width
DIAG_OFF = [0, 128, 256, 256]
DIAG_W = [512, 384, 256, 256]

_prog_cache = {}


def _build(shared_freqs: bool, repeat: int = 1, timing: bool = False,
           unit_w: bool = True):
    import concourse.bacc as bacc
    import concourse.mybir as mybir
    import concourse.tile as tile

    dt = mybir.dt
    f32 = dt.float32
    AF = mybir.ActivationFunctionType
    mode = MM_DTYPE

    nc = bacc.Bacc("TRN2", target_bir_lowering=False, debug=False,
                   num_devices=NCORES)
    mdt = {"f32": f32, "f32r": dt.float32r, "bf16": dt.bfloat16}[mode]
    # transpose-path dtype (qr, eye): bf16 transposes at 1 cyc/row; f32 at 2.
    # (f32r transpose compiles but breaks on hardware -- keep f32.)
    tdt = dt.bfloat16 if mode == "bf16" else f32
    # normalize path (denom reciprocal -> broadcast matmul) stays 4-byte
    ndt = dt.float32r if mode != "f32" else f32
    ikind = "Internal" if timing else "ExternalInput"
    okind = "Internal" if timing else "ExternalOutput"
    xT_d = nc.dram_tensor("xT", [D, T], mdt, kind=ikind).ap()
    wq_d = nc.dram_tensor("wq", [D, GW], mdt, kind=ikind).ap()
    wkv_d = nc.dram_tensor("wkv", [D, 2 * HD], mdt, kind=ikind).ap()
    wo_d = nc.dram_tensor("wo", [GW, D], mdt, kind=ikind).ap()
    nf = 256 if shared_freqs else 512
    f8_d = nc.dram_tensor("f8", [T, nf], f32, kind=ikind).ap()
    msk_d = nc.dram_tensor("msk", [128, 128], f32, kind=ikind).ap()
    eye_d = nc.dram_tensor("eye", [128, 128], tdt, kind=ikind).ap()
    onc_d = nc.dram_tensor("onc", [128, 1], mdt, kind=ikind).ap()
    onr_d = nc.dram_tensor("onr", [1, 128], ndt, kind=ikind).ap()
    y_d = nc.dram_tensor("y", [T, D], f32, kind=okind).ap()
    if timing:
        din = nc.dram_tensor("din", [128, 4], f32, kind="ExternalInput").ap()
        dout = nc.dram_tensor("dout", [128, 4], f32,
                              kind="ExternalOutput").ap()

    def mm(out, lhsT, rhs, start, stop):
        nc.tensor.matmul(out, lhsT, rhs, start=start, stop=stop,
                         skip_group_check=True)

    with nc.allow_low_precision(reason="f32r tiles feed full-rate matmuls"), \
         tile.TileContext(nc) as tc:
        with tc.tile_pool(name="const", bufs=1) as cpool, \
             tc.tile_pool(name="resid", bufs=1) as rpool:
            if timing:
                dsb = cpool.tile([128, 4], f32)
                nc.sync.dma_start(dsb[:], din[:])
                nc.sync.dma_start(dout[:], dsb[:])
            msk_sb = cpool.tile([128, 128], f32)
            nc.sync.dma_start(msk_sb[:], msk_d[:])
            eye_sb = cpool.tile([128, 128], tdt)
            nc.sync.dma_start(eye_sb[:], eye_d[:])
            onc_sb = cpool.tile([128, 1], mdt)
            nc.sync.dma_start(onc_sb[:], onc_d[:])
            onr_sb = cpool.tile([1, 128], ndt)
            nc.sync.dma_start(onr_sb[:], onr_d[:])
            eps_sb = cpool.tile([128, 1], f32)
            nc.vector.memset(eps_sb[:], EPS)

            # residents: qT/kT per head_dim-major, v natural, oT per head
            qT = rpool.tile([128, NH * T], mdt)    # head h at [:, h*T : (h+1)*T]
            kT = rpool.tile([128, T], mdt)
            vv = rpool.tile([128, T], mdt)         # chunk j at [:, j*128 : ...]
            oT = rpool.tile([128, NH * T], mdt)

            for _rep in range(repeat):
                # ---------------- phase A: projections + rmsnorm + rope ---------
                with tc.tile_pool(name=f"wA{_rep}", bufs=1) as wA, \
                     tc.tile_pool(name=f"xs{_rep}", bufs=2) as xsp, \
                     tc.tile_pool(name=f"fA{_rep}", bufs=2) as fap, \
                     tc.tile_pool(name=f"qrp{_rep}", bufs=2) as qrp, \
                     tc.tile_pool(name=f"smA{_rep}", bufs=2) as smp, \
                     tc.tile_pool(name=f"psA{_rep}", bufs=2, space="PSUM") as psA, \
                     tc.tile_pool(name=f"psK{_rep}", bufs=2, space="PSUM") as psK, \
                     tc.tile_pool(name=f"psT{_rep}", bufs=2, space="PSUM") as psT:
                    wq_sb = wA.tile([128, DC * GW], mdt)
                    wkv_sb = wA.tile([128, DC * 2 * HD], mdt)
                    wqr = wq_sb.rearrange("p (c n) -> p c n", c=DC)
                    wqs = wq_d.rearrange("(c p) n -> p c n", p=128)
                    wkr = wkv_sb.rearrange("p (c n) -> p c n", c=DC)
                    wks = wkv_d.rearrange("(c p) n -> p c n", p=128)
                    for c in range(DC):
                        nc.sync.dma_start(wqr[:, c, :], wqs[:, c, :])
                        nc.sync.dma_start(wkr[:, c, :], wks[:, c, :])
                    wq_v = wq_sb.rearrange("p (c n) -> p c n", c=DC)
                    wkv_v = wkv_sb.rearrange("p (c n) -> p c n", c=DC)

                    xs = None
                    pend_qr = None
                    for t in range(TT):
                        g2, half = divmod(t, 2)
                        if half == 0:
                            xs = xsp.tile([128, DC * 256], mdt, name="xs")
                            xsr = xs.rearrange("p (c n) -> p c n", c=DC)
                            xss = xT_d.rearrange("(c p) n -> p c n", p=128)
                            for c in range(DC):
                                nc.sync.dma_start(
                                    xsr[:, c, :],
                                    xss[:, c, g2 * 256:(g2 + 1) * 256])
                        xs_v = xs.rearrange("p (c n) -> p c n", c=DC)
                        f8t = fap.tile([128, nf], f32, name="f8t")
                        nc.sync.dma_start(f8t[:], f8_d[t * 128:(t + 1) * 128, :])

                        q_ps = psA.tile([128, GW], f32, name="q_ps")
                        kv_ps = psK.tile([128, 2 * HD], f32, name="kv_ps")
                        for c in range(DC):
                            mm(q_ps[:], xs_v[:, c, half * 128:(half + 1) * 128],
                               wq_v[:, c, :], c == 0, c == DC - 1)
                        for c in range(DC):
                            mm(kv_ps[:], xs_v[:, c, half * 128:(half + 1) * 128],
                               wkv_v[:, c, :], c == 0, c == DC - 1)

                        # v: natural layout, straight into the resident tile
                        nc.scalar.copy(vv[:, t * 128:(t + 1) * 128],
                                       kv_ps[:, HD:2 * HD])

                        # rmsnorm sums of squares per head. When the norm
                        # weights are ones, rope is a pure rotation so the
                        # sums can be taken from the rope output qr on DVE
                        # (emitted after rope below); otherwise from the raw
                        # projections via ACT Square+accum.
                        ssq = smp.tile([128, 8], f32, name="ssq")
                        if not unit_w:
                            sqs = smp.tile([128, 128], f32, name="sqs")
                            for h5 in range(5):
                                src = (q_ps[:, h5 * 128:(h5 + 1) * 128]
                                       if h5 < 4 else kv_ps[:, 0:HD])
                                nc.scalar.activation(sqs[:], src, AF.Square,
                                                     accum_out=ssq[:,
                                                                   h5:h5 + 1])

                        # rope (freqs carry the rmsnorm weights); qr holds the 4 q
                        # heads then k, all [evens|odds] within each 128 block
                        qr = qrp.tile([128, 640], tdt, name="qr")
                        t1 = qrp.tile([128, 256], f32, name="t1")
                        t2 = qrp.tile([128, 256], f32, name="t2")
                        q_v = q_ps.rearrange("p (h x) -> p h x", h=4)
                        qe, qo = q_v[:, :, 0:64], q_v[:, :, 64:128]
                        t1_v = t1.rearrange("p (h x) -> p h x", h=4)
                        t2_v = t2.rearrange("p (h x) -> p h x", h=4)
                        qr_v = qr.rearrange("p (h x) -> p h x", h=5)

                        def fq(k4):  # freq slice broadcast over the 4 q heads
                            s = f8t[:, k4 * 64:(k4 + 1) * 64]
                            return s.rearrange("p (o x) -> p o x", o=1) \
                                    .broadcast_to([128, 4, 64])

                        nc.vector.tensor_mul(t1_v, qe, fq(0))          # e*cosE
                        nc.vector.tensor_mul(t2_v, qo, fq(1))          # o*sinO
                        nc.vector.tensor_sub(qr_v[:, 0:4, 0:64], t1_v, t2_v)
                        nc.vector.tensor_mul(t1_v, qe, fq(2))          # e*sinE
                        nc.vector.tensor_mul(t2_v, qo, fq(3))          # o*cosO
                        nc.vector.tensor_add(qr_v[:, 0:4, 64:128], t1_v, t2_v)

                        kf0 = 0 if shared_freqs else 4
                        ke, ko = kv_ps[:, 0:64], kv_ps[:, 64:128]
                        kt1 = smp.tile([128, 64], f32, name="kt1")
                        kt2 = smp.tile([128, 64], f32, name="kt2")

                        def fk(k4):
                            return f8t[:, (kf0 + k4) * 64:(kf0 + k4 + 1) * 64]

                        nc.vector.tensor_mul(kt1[:], ke, fk(0))
                        nc.vector.tensor_mul(kt2[:], ko, fk(1))
                        nc.vector.tensor_sub(qr[:, 512:576], kt1[:], kt2[:])
                        nc.vector.tensor_mul(kt1[:], ke, fk(2))
                        nc.vector.tensor_mul(kt2[:], ko, fk(3))
                        nc.vector.tensor_add(qr[:, 576:640], kt1[:], kt2[:])

                        if unit_w:
                            sqs = smp.tile([128, 128], f32, name="sqs")
                            for h5 in range(5):
                                sl = qr[:, h5 * 128:(h5 + 1) * 128]
                                slf = sl.bitcast(f32)
                                nc.vector.scalar_tensor_tensor(
                                    sqs[:], slf, 1.0, slf,
                                    mybir.AluOpType.mult,
                                    mybir.AluOpType.mult,
                                    accum_out=ssq[:, h5:h5 + 1])
                        rstd = smp.tile([128, 8], f32, name="rstd")
                        nc.scalar.activation(rstd[:, 0:5], ssq[:, 0:5], AF.Sqrt,
                                             bias=eps_sb[:], scale=1.0 / HD)
                        rms = smp.tile([128, 8], f32, name="rms")
                        nc.vector.reciprocal(rms[:, 0:5], rstd[:, 0:5])
                        for h5 in range(5):
                            sl = qr[:, h5 * 128:(h5 + 1) * 128]
                            nc.vector.tensor_scalar_mul(sl, sl, rms[:, h5:h5 + 1])

                        # transpose each head block into the resident qT /
                        # kT -- deferred one ttile so the PE never waits on
                        # the DVE rope chain (software pipeline).
                        if pend_qr is not None:
                            pqr, pt = pend_qr
                            for h5 in range(5):
                                tp_ps = psT.tile([128, 128], tdt, name="tp_ps")
                                nc.tensor.transpose(
                                    tp_ps[:], pqr[:, h5 * 128:(h5 + 1) * 128],
                                    eye_sb[:])
                                dst = (qT[:, h5 * T + pt * 128:
                                          h5 * T + (pt + 1) * 128]
                                       if h5 < 4
                                       else kT[:, pt * 128:(pt + 1) * 128])
                                nc.scalar.copy(dst, tp_ps[:])
                        pend_qr = (qr, t)

                        pqr, pt = pend_qr
                    for h5 in range(5):
                        tp_ps = psT.tile([128, 128], tdt, name="tp_ps")
                        nc.tensor.transpose(
                            tp_ps[:], pqr[:, h5 * 128:(h5 + 1) * 128],
                            eye_sb[:])
                        dst = (qT[:, h5 * T + pt * 128:
                                  h5 * T + (pt + 1) * 128]
                               if h5 < 4 else kT[:, pt * 128:(pt + 1) * 128])
                        nc.scalar.copy(dst, tp_ps[:])

                # ---------------- phase B: attention ---------------------------
                with tc.tile_pool(name=f"attp{_rep}", bufs=3) as attp, \
                     tc.tile_pool(name=f"smB{_rep}", bufs=2) as smB, \
                     tc.tile_pool(name=f"psS{_rep}", bufs=3, space="PSUM") as psS, \
                     tc.tile_pool(name=f"psO{_rep}", bufs=2, space="PSUM") as psO, \
                     tc.tile_pool(name=f"psD{_rep}", bufs=2, space="PSUM") as psD, \
                     tc.tile_pool(name=f"psB{_rep}", bufs=1, space="PSUM") as psB:
                    def flush_norm(pn):
                        po, pd, ph, pi = pn
                        rec = smB.tile([1, 512], ndt, name="rec")
                        nc.vector.reciprocal(rec[:], pd[:])
                        bc_ps = psB.tile([128, 512], f32, name="bc_ps")
                        mm(bc_ps[:], onr_sb[:], rec[:], True, True)
                        osl = oT[:, ph * T + pi * 512:
                                 ph * T + (pi + 1) * 512]
                        nc.scalar.copy(osl, po[:])
                        nc.vector.tensor_mul(osl, osl, bc_ps[:])

                    pend_norm = None
                    for h in range(NH):
                        for i in range(NI):
                            o_ps = psO.tile([128, 512], f32, name="o_ps")
                            d_ps = psD.tile([1, 512], f32, name="d_ps")
                            nj = 4 * i + 4
                            pend_att = None
                            for j in range(nj):
                                # previous tile's softmax normalization runs
                                # inside this tile's chunk stream so the PE
                                # isn't stalled on the DVE reciprocal
                                if j == 1 and pend_norm is not None:
                                    flush_norm(pend_norm)
                                    pend_norm = None
                                r = j - 4 * i
                                off = DIAG_OFF[r] if r >= 0 else 0
                                w = DIAG_W[r] if r >= 0 else 512
                                s_ps = psS.tile([128, 512], f32, name="s_ps")
                                mm(s_ps[:, 0:w], kT[:, j * 128:(j + 1) * 128],
                                   qT[:, h * T + i * 512 + off:
                                       h * T + i * 512 + off + w], True, True)
                                att = attp.tile([128, 512], mdt, name="att")
                                if r >= 0:
                                    mr = 128 if r == 3 else 0
                                    nc.vector.tensor_add(
                                        s_ps[:, mr:mr + 128],
                                        s_ps[:, mr:mr + 128], msk_sb[:])
                                    if r == 3:
                                        zdt = (f32 if dt.size(mdt) == 4
                                               else dt.uint16)
                                        nc.gpsimd.memset(
                                            att[:, 0:128].bitcast(zdt), 0)
                                        nc.scalar.activation(
                                            att[:, 128:256], s_ps[:, 128:256],
                                            AF.Exp, scale=SCALE)
                                    else:
                                        nc.scalar.activation(
                                            att[:, 0:w], s_ps[:, 0:w],
                                            AF.Exp, scale=SCALE)
                                else:
                                    nc.scalar.activation(att[:, 0:w], s_ps[:, 0:w],
                                                         AF.Exp, scale=SCALE)
                                if pend_att is not None:
                                    patt, poff, pw, pj = pend_att
                                    mm(o_ps[:, poff:poff + pw],
                                       vv[:, pj * 128:(pj + 1) * 128],
                                       patt[:, 0:pw], pj == 0, False)
                                    mm(d_ps[:, poff:poff + pw], onc_sb[:],
                                       patt[:, 0:pw], pj == 0, False)
                                pend_att = (att, off, w, j)
                            patt, poff, pw, pj = pend_att
                            mm(o_ps[:, poff:poff + pw],
                               vv[:, pj * 128:(pj + 1) * 128],
                               patt[:, 0:pw], pj == 0, True)
                            mm(d_ps[:, poff:poff + pw], onc_sb[:],
                               patt[:, 0:pw], pj == 0, True)
                            pend_norm = (o_ps, d_ps, h, i)
                    flush_norm(pend_norm)

                # ---------------- phase C: output projection --------------------
                with tc.tile_pool(name=f"woC{_rep}", bufs=3) as wop, \
                     tc.tile_pool(name=f"ysb{_rep}", bufs=6) as ysb, \
                     tc.tile_pool(name=f"psY{_rep}", bufs=4, space="PSUM") as psY:
                    for d in range(4):
                        wo_sb = wop.tile([128, 4 * 512], mdt, name="wo_sb")
                        wor = wo_sb.rearrange("p (c n) -> p c n", c=4)
                        wos = wo_d.rearrange("(c p) n -> p c n", p=128)
                        for c in range(4):
                            nc.sync.dma_start(
                                wor[:, c, :],
                                wos[:, c, d * 512:(d + 1) * 512])
                        for t in range(TT):
                            y_ps = psY.tile([128, 512], f32, name="y_ps")
                            for lc in range(4):
                                mm(y_ps[:],
                                   oT[:, lc * T + t * 128: lc * T + (t + 1) * 128],
                                   wo_sb[:, lc * 512:(lc + 1) * 512],
                                   lc == 0, lc == 3)
                            y_sb = ysb.tile([128, 512], f32, name="y_sb")
                            if (d * TT + t) % 2 == 0:
                                nc.scalar.copy(y_sb[:], y_ps[:])
                            else:
                                nc.vector.tensor_copy(y_sb[:], y_ps[:])
                            nc.sync.dma_start(
                                y_d[t * 128:(t + 1) * 128, d * 512:(d + 1) * 512],
                                y_sb[:])

    nc.compile()
    return nc


_EVOD = None


def _perm():
    global _EVOD
    if _EVOD is None:
        _EVOD = np.concatenate([np.arange(0, HD, 2), np.arange(1, HD, 2)])
    return _EVOD


def prepare_inputs(x, wq, wk, wv, wo, q_norm_w, k_norm_w, freqs_cos, freqs_sin):
    """Host-side sharding + layout prep. Returns (in_maps, shared_freqs)."""
    x = np.asarray(x, np.float32)
    wq = np.asarray(wq, np.float32)
    wk = np.asarray(wk, np.float32)
    wv = np.asarray(wv, np.float32)
    wo = np.asarray(wo, np.float32)
    qw = np.asarray(q_norm_w, np.float32)
    kw = np.asarray(k_norm_w, np.float32)
    cos = np.asarray(freqs_cos, np.float32)
    sin = np.asarray(freqs_sin, np.float32)

    perm = _perm()
    shared = bool(np.allclose(qw, kw))
    unit_w = bool(np.allclose(qw, 1.0) and np.allclose(kw, 1.0))

    def freq4(w):
        we, wo_ = w[0::2], w[1::2]
        return np.concatenate(
            [cos * we[None, :], sin * wo_[None, :],
             sin * we[None, :], cos * wo_[None, :]], axis=1)

    f8 = freq4(qw) if shared else np.concatenate([freq4(qw), freq4(kw)], axis=1)
    f8 = np.ascontiguousarray(f8, np.float32)

    msk = np.where(np.arange(128)[None, :] >= np.arange(128)[:, None],
                   np.float32(0.0), np.float32(NEG)).astype(np.float32)
    eye = np.eye(128, dtype=np.float32)
    onc = np.ones((128, 1), np.float32)
    onr = np.ones((1, 128), np.float32)

    if MM_DTYPE == "bf16":
        import ml_dtypes
        mnp = ml_dtypes.bfloat16
    else:
        mnp = np.float32
    tnp = mnp if MM_DTYPE == "bf16" else np.float32

    xTs = [np.ascontiguousarray(x[b].T).astype(mnp) for b in range(B)]
    eye = eye.astype(tnp)
    onc = onc.astype(mnp)
    in_maps = []
    for c in range(NCORES):
        b, g = divmod(c, KV)
        wq_g = wq[:, g * GW:(g + 1) * GW].reshape(D, NH, HD)[:, :, perm] \
            .reshape(D, GW)
        wk_g = wk[:, g * HD:(g + 1) * HD][:, perm]
        wv_g = wv[:, g * HD:(g + 1) * HD]
        wkv_g = np.ascontiguousarray(
            np.concatenate([wk_g, wv_g], axis=1)).astype(mnp)
        wo_g = np.ascontiguousarray(wo[g * GW:(g + 1) * GW, :]).astype(mnp)
        in_maps.append(dict(
            xT=xTs[b], wq=np.ascontiguousarray(wq_g).astype(mnp),
            wkv=wkv_g, wo=wo_g, f8=f8, msk=msk, eye=eye, onc=onc, onr=onr))
    return in_maps, shared, unit_w


def get_program(shared_freqs: bool, repeat: int = 1, timing: bool = False,
                unit_w: bool = True):
    key = (shared_freqs, MM_DTYPE, repeat, timing, unit_w)
    if key not in _prog_cache:
        _prog_cache[key] = _build(shared_freqs, repeat, timing, unit_w)
    return _prog_cache[key]


def kernel(**inputs):
    from concourse.bass_utils import run_bass_kernel_spmd

    in_maps, shared, unit_w = prepare_inputs(**inputs)
    nc = get_program(shared, unit_w=unit_w)
    res = run_bass_kernel_spmd(nc, in_maps, list(range(NCORES)))
    out = np.empty((B, T, D), np.float32)
    for b in range(B):
        acc = res.results[b * KV + 0]["y"].astype(np.float32)
        for g in range(1, KV):
            acc = acc + res.results[b * KV + g]["y"]
        out[b] = acc
    return out



# revision 3
# speedup vs baseline: 1.0649x; 1.0649x over previous
"""Trainium2 Bass kernel for nn_Attention_26182120636812 (GQA attention block).

Sharding: 8 cores = 2 (batch) x 4 (KV groups). Each core computes, for its
batch element b and kv-group g: the 4 query heads + 1 kv head of group g,
full causal attention over T=2048, and the partial output projection
y_part = o_g @ wo[g*512:(g+1)*512, :]. The host sums the 4 partials per batch.

Device-side layout choices (see comments inline):
 - host passes x[b]^T so projection matmuls need no on-device transpose of x
 - head_dim is permuted host-side to [evens|odds] so RoPE works on contiguous
   halves; the permutation cancels in q.k and is never visible in the output
 - rmsnorm weights are folded into the rope cos/sin tensors host-side
 - scores are computed transposed (tk x tq) so softmax (no max subtraction --
   logits are bounded by sqrt(HD)=11.3 after rmsnorm) needs no transposes:
   exp on ACT, denominators via ones-matmul, 1/denom broadcast via K=1 matmul
 - causal masking: block skipping + split diagonal chunks + one 128x128 mask
 - matmuls run as float32r (full PE rate at N>=256); set MM_F32R=False for
   exact fp32 (4x slower on PE)
"""
import sys

for _p in ("/opt/trn_rl_repo",):
    if _p not in sys.path:
        sys.path.insert(0, _p)

import numpy as np

B, T, D = 2, 2048, 2048
H, KV, HD = 16, 4, 128
NCORES = 8
NH = H // KV          # 4 q heads per core
GW = NH * HD          # 512: per-core q / o width
TT = T // 128         # 16 token tiles
NI = T // 512         # 4 query super-tiles
DC = D // 128         # 16 contraction chunks over D
EPS = 1e-6
SCALE = 1.0 / float(np.sqrt(HD))
NEG = -1.0e30
MM_DTYPE = "bf16"      # "f32" | "f32r" | "bf16"

# diagonal chunk r in 0..3 of a 512-wide query tile: computed tq offset/width
DIAG_OFF = [0, 128, 256, 256]
DIAG_W = [512, 384, 256, 256]

_prog_cache = {}


def _build(shared_freqs: bool, repeat: int = 1, timing: bool = False,
           unit_w: bool = True):
    import concourse.bacc as bacc
    import concourse.mybir as mybir
    import concourse.tile as tile

    dt = mybir.dt
    f32 = dt.float32
    AF = mybir.ActivationFunctionType
    mode = MM_DTYPE

    nc = bacc.Bacc("TRN2", target_bir_lowering=False, debug=False,
                   num_devices=NCORES)
    mdt = {"f32": f32, "f32r": dt.float32r, "bf16": dt.bfloat16}[mode]
    # transpose-path dtype (qr, eye): bf16 transposes at 1 cyc/row; f32 at 2.
    # (f32r transpose compiles but breaks on hardware -- keep f32.)
    tdt = dt.bfloat16 if mode == "bf16" else f32
    # normalize path (denom reciprocal -> broadcast matmul) stays 4-byte
    ndt = dt.float32r if mode != "f32" else f32
    ikind = "Internal" if timing else "ExternalInput"
    okind = "Internal" if timing else "ExternalOutput"
    xT_d = nc.dram_tensor("xT", [D, T], mdt, kind=ikind).ap()
    wq_d = nc.dram_tensor("wq", [D, GW], mdt, kind=ikind).ap()
    wkv_d = nc.dram_tensor("wkv", [D, 2 * HD], mdt, kind=ikind).ap()
    wo_d = nc.dram_tensor("wo", [GW, D], mdt, kind=ikind).ap()
    nf = 256 if shared_freqs else 512
    f8_d = nc.dram_tensor("f8", [T, nf], f32, kind=ikind).ap()
    msk_d = nc.dram_tensor("msk", [128, 128], f32, kind=ikind).ap()
    eye_d = nc.dram_tensor("eye", [128, 128], tdt, kind=ikind).ap()
    onc_d = nc.dram_tensor("onc", [128, 1], mdt, kind=ikind).ap()
    onr_d = nc.dram_tensor("onr", [1, 128], ndt, kind=ikind).ap()
    y_d = nc.dram_tensor("y", [T, D], f32, kind=okind).ap()
    if timing:
        din = nc.dram_tensor("din", [128, 4], f32, kind="ExternalInput").ap()
        dout = nc.dram_tensor("dout", [128, 4], f32,
                              kind="ExternalOutput").ap()

    def mm(out, lhsT, rhs, start, stop):
        nc.tensor.matmul(out, lhsT, rhs, start=start, stop=stop,
                         skip_group_check=True)

    with nc.allow_low_precision(reason="f32r tiles feed full-rate matmuls"), \
         tile.TileContext(nc) as tc:
        with tc.tile_pool(name="const", bufs=1) as cpool, \
             tc.tile_pool(name="resid", bufs=1) as rpool:
            if timing:
                dsb = cpool.tile([128, 4], f32)
                nc.sync.dma_start(dsb[:], din[:])
                nc.sync.dma_start(dout[:], dsb[:])
            msk_sb = cpool.tile([128, 128], f32)
            nc.sync.dma_start(msk_sb[:], msk_d[:])
            eye_sb = cpool.tile([128, 128], tdt)
            nc.sync.dma_start(eye_sb[:], eye_d[:])
            onc_sb = cpool.tile([128, 1], mdt)
            nc.sync.dma_start(onc_sb[:], onc_d[:])
            onr_sb = cpool.tile([1, 128], ndt)
            nc.sync.dma_start(onr_sb[:], onr_d[:])
            eps_sb = cpool.tile([128, 1], f32)
            nc.vector.memset(eps_sb[:], EPS)

            # residents: qT/kT per head_dim-major, v natural, oT per head
            qT = rpool.tile([128, NH * T], mdt)    # head h at [:, h*T : (h+1)*T]
            kT = rpool.tile([128, T], mdt)
            vv = rpool.tile([128, T], mdt)         # chunk j at [:, j*128 : ...]
            oT = rpool.tile([128, NH * T], mdt)

            for _rep in range(repeat):
                # ---------------- phase A: projections + rmsnorm + rope ---------
                with tc.tile_pool(name=f"wA{_rep}", bufs=1) as wA, \
                     tc.tile_pool(name=f"xs{_rep}", bufs=2) as xsp, \
                     tc.tile_pool(name=f"fA{_rep}", bufs=2) as fap, \
                     tc.tile_pool(name=f"qrp{_rep}", bufs=2) as qrp, \
                     tc.tile_pool(name=f"smA{_rep}", bufs=2) as smp, \
                     tc.tile_pool(name=f"psA{_rep}", bufs=2, space="PSUM") as psA, \
                     tc.tile_pool(name=f"psK{_rep}", bufs=2, space="PSUM") as psK, \
                     tc.tile_pool(name=f"psT{_rep}", bufs=2, space="PSUM") as psT:
                    wq_sb = wA.tile([128, DC * GW], mdt)
                    wkv_sb = wA.tile([128, DC * 2 * HD], mdt)
                    wqr = wq_sb.rearrange("p (c n) -> p c n", c=DC)
                    wqs = wq_d.rearrange("(c p) n -> p c n", p=128)
                    wkr = wkv_sb.rearrange("p (c n) -> p c n", c=DC)
                    wks = wkv_d.rearrange("(c p) n -> p c n", p=128)
                    for c in range(DC):
                        nc.sync.dma_start(wqr[:, c, :], wqs[:, c, :])
                        nc.sync.dma_start(wkr[:, c, :], wks[:, c, :])
                    wq_v = wq_sb.rearrange("p (c n) -> p c n", c=DC)
                    wkv_v = wkv_sb.rearrange("p (c n) -> p c n", c=DC)

                    xs = None
                    pend_qr = None
                    for t in range(TT):
                        g2, half = divmod(t, 2)
                        if half == 0:
                            xs = xsp.tile([128, DC * 256], mdt, name="xs")
                            xsr = xs.rearrange("p (c n) -> p c n", c=DC)
                            xss = xT_d.rearrange("(c p) n -> p c n", p=128)
                            for c in range(DC):
                                nc.sync.dma_start(
                                    xsr[:, c, :],
                                    xss[:, c, g2 * 256:(g2 + 1) * 256])
                        xs_v = xs.rearrange("p (c n) -> p c n", c=DC)
                        f8t = fap.tile([128, nf], f32, name="f8t")
                        nc.sync.dma_start(f8t[:], f8_d[t * 128:(t + 1) * 128, :])

                        q_ps = psA.tile([128, GW], f32, name="q_ps")
                        kv_ps = psK.tile([128, 2 * HD], f32, name="kv_ps")
                        for c in range(DC):
                            mm(q_ps[:], xs_v[:, c, half * 128:(half + 1) * 128],
                               wq_v[:, c, :], c == 0, c == DC - 1)
                        for c in range(DC):
                            mm(kv_ps[:], xs_v[:, c, half * 128:(half + 1) * 128],
                               wkv_v[:, c, :], c == 0, c == DC - 1)

                        # v: natural layout, straight into the resident tile
                        nc.scalar.copy(vv[:, t * 128:(t + 1) * 128],
                                       kv_ps[:, HD:2 * HD])

                        # rmsnorm sums of squares per head. When the norm
                        # weights are ones, rope is a pure rotation so the
                        # sums can be taken from the rope output qr on DVE
                        # (emitted after rope below); otherwise from the raw
                        # projections via ACT Square+accum.
                        ssq = smp.tile([128, 8], f32, name="ssq")
                        if not unit_w:
                            sqs = smp.tile([128, 128], f32, name="sqs")
                            for h5 in range(5):
                                src = (q_ps[:, h5 * 128:(h5 + 1) * 128]
                                       if h5 < 4 else kv_ps[:, 0:HD])
                                nc.scalar.activation(sqs[:], src, AF.Square,
                                                     accum_out=ssq[:,
                                                                   h5:h5 + 1])

                        # rope (freqs carry the rmsnorm weights); qr holds the 4 q
                        # heads then k, all [evens|odds] within each 128 block
                        qr = qrp.tile([128, 640], tdt, name="qr")
                        t1 = qrp.tile([128, 256], f32, name="t1")
                        t2 = qrp.tile([128, 256], f32, name="t2")
                        q_v = q_ps.rearrange("p (h x) -> p h x", h=4)
                        qe, qo = q_v[:, :, 0:64], q_v[:, :, 64:128]
                        t1_v = t1.rearrange("p (h x) -> p h x", h=4)
                        t2_v = t2.rearrange("p (h x) -> p h x", h=4)
                        qr_v = qr.rearrange("p (h x) -> p h x", h=5)

                        def fq(k4):  # freq slice broadcast over the 4 q heads
                            s = f8t[:, k4 * 64:(k4 + 1) * 64]
                            return s.rearrange("p (o x) -> p o x", o=1) \
                                    .broadcast_to([128, 4, 64])

                        nc.vector.tensor_mul(t1_v, qe, fq(0))          # e*cosE
                        nc.vector.tensor_mul(t2_v, qo, fq(1))          # o*sinO
                        nc.vector.tensor_sub(qr_v[:, 0:4, 0:64], t1_v, t2_v)
                        nc.vector.tensor_mul(t1_v, qe, fq(2))          # e*sinE
                        nc.vector.tensor_mul(t2_v, qo, fq(3))          # o*cosO
                        nc.vector.tensor_add(qr_v[:, 0:4, 64:128], t1_v, t2_v)

                        kf0 = 0 if shared_freqs else 4
                        ke, ko = kv_ps[:, 0:64], kv_ps[:, 64:128]
                        kt1 = smp.tile([128, 64], f32, name="kt1")
                        kt2 = smp.tile([128, 64], f32, name="kt2")

                        def fk(k4):
                            return f8t[:, (kf0 + k4) * 64:(kf0 + k4 + 1) * 64]

                        nc.vector.tensor_mul(kt1[:], ke, fk(0))
                        nc.vector.tensor_mul(kt2[:], ko, fk(1))
                        nc.vector.tensor_sub(qr[:, 512:576], kt1[:], kt2[:])
                        nc.vector.tensor_mul(kt1[:], ke, fk(2))
                        nc.vector.tensor_mul(kt2[:], ko, fk(3))
                        nc.vector.tensor_add(qr[:, 576:640], kt1[:], kt2[:])

                        if unit_w:
                            sqs = smp.tile([128, 128], f32, name="sqs")
                            for h5 in range(5):
                                sl = qr[:, h5 * 128:(h5 + 1) * 128]
                                slf = sl.bitcast(f32) if dt.size(tdt) == 4 \
                                    else sl
                                nc.vector.scalar_tensor_tensor(
                                    sqs[:], slf, 1.0, slf,
                                    mybir.AluOpType.mult,
                                    mybir.AluOpType.mult,
                                    accum_out=ssq[:, h5:h5 + 1])
                        rstd = smp.tile([128, 8], f32, name="rstd")
                        nc.scalar.activation(rstd[:, 0:5], ssq[:, 0:5], AF.Sqrt,
                                             bias=eps_sb[:], scale=1.0 / HD)
                        rms = smp.tile([128, 8], f32, name="rms")
                        nc.vector.reciprocal(rms[:, 0:5], rstd[:, 0:5])
                        for h5 in range(5):
                            sl = qr[:, h5 * 128:(h5 + 1) * 128]
                            nc.vector.tensor_scalar_mul(sl, sl, rms[:, h5:h5 + 1])

                        # transpose each head block into the resident qT /
                        # kT -- deferred one ttile so the PE never waits on
                        # the DVE rope chain (software pipeline).
                        if pend_qr is not None:
                            pqr, pt = pend_qr
                            for h5 in range(5):
                                tp_ps = psT.tile([128, 128], tdt, name="tp_ps")
                                nc.tensor.transpose(
                                    tp_ps[:], pqr[:, h5 * 128:(h5 + 1) * 128],
                                    eye_sb[:])
                                dst = (qT[:, h5 * T + pt * 128:
                                          h5 * T + (pt + 1) * 128]
                                       if h5 < 4
                                       else kT[:, pt * 128:(pt + 1) * 128])
                                nc.scalar.copy(dst, tp_ps[:])
                        pend_qr = (qr, t)

                        pqr, pt = pend_qr
                    for h5 in range(5):
                        tp_ps = psT.tile([128, 128], tdt, name="tp_ps")
                        nc.tensor.transpose(
                            tp_ps[:], pqr[:, h5 * 128:(h5 + 1) * 128],
                            eye_sb[:])
                        dst = (qT[:, h5 * T + pt * 128:
                                  h5 * T + (pt + 1) * 128]
                               if h5 < 4 else kT[:, pt * 128:(pt + 1) * 128])
                        nc.scalar.copy(dst, tp_ps[:])

                # ---------------- phase B: attention ---------------------------
                with tc.tile_pool(name=f"attp{_rep}", bufs=3) as attp, \
                     tc.tile_pool(name=f"smB{_rep}", bufs=2) as smB, \
                     tc.tile_pool(name=f"psS{_rep}", bufs=3, space="PSUM") as psS, \
                     tc.tile_pool(name=f"psO{_rep}", bufs=2, space="PSUM") as psO, \
                     tc.tile_pool(name=f"psD{_rep}", bufs=2, space="PSUM") as psD, \
                     tc.tile_pool(name=f"psB{_rep}", bufs=1, space="PSUM") as psB:
                    def flush_norm(pn):
                        po, pd, ph, pi = pn
                        rec = smB.tile([1, 512], ndt, name="rec")
                        nc.vector.reciprocal(rec[:], pd[:])
                        bc_ps = psB.tile([128, 512], f32, name="bc_ps")
                        mm(bc_ps[:], onr_sb[:], rec[:], True, True)
                        osl = oT[:, ph * T + pi * 512:
                                 ph * T + (pi + 1) * 512]
                        nc.scalar.copy(osl, po[:])
                        nc.vector.tensor_mul(osl, osl, bc_ps[:])

                    pend_norm = None
                    for h in range(NH):
                        for i in range(NI):
                            o_ps = psO.tile([128, 512], f32, name="o_ps")
                            d_ps = psD.tile([1, 512], f32, name="d_ps")
                            nj = 4 * i + 4
                            pend_att = None
                            for j in range(nj):
                                # previous tile's softmax normalization runs
                                # inside this tile's chunk stream so the PE
                                # isn't stalled on the DVE reciprocal
                                if j == 1 and pend_norm is not None:
                                    flush_norm(pend_norm)
                                    pend_norm = None
                                r = j - 4 * i
                                off = DIAG_OFF[r] if r >= 0 else 0
                                w = DIAG_W[r] if r >= 0 else 512
                                s_ps = psS.tile([128, 512], f32, name="s_ps")
                                mm(s_ps[:, 0:w], kT[:, j * 128:(j + 1) * 128],
                                   qT[:, h * T + i * 512 + off:
                                       h * T + i * 512 + off + w], True, True)
                                att = attp.tile([128, 512], mdt, name="att")
                                if r >= 0:
                                    mr = 128 if r == 3 else 0
                                    nc.vector.tensor_add(
                                        s_ps[:, mr:mr + 128],
                                        s_ps[:, mr:mr + 128], msk_sb[:])
                                    if r == 3:
                                        zdt = (f32 if dt.size(mdt) == 4
                                               else dt.uint16)
                                        nc.gpsimd.memset(
                                            att[:, 0:128].bitcast(zdt), 0)
                                        nc.scalar.activation(
                                            att[:, 128:256], s_ps[:, 128:256],
                                            AF.Exp, scale=SCALE)
                                    else:
                                        nc.scalar.activation(
                                            att[:, 0:w], s_ps[:, 0:w],
                                            AF.Exp, scale=SCALE)
                                else:
                                    nc.scalar.activation(att[:, 0:w], s_ps[:, 0:w],
                                                         AF.Exp, scale=SCALE)
                                if pend_att is not None:
                                    patt, poff, pw, pj = pend_att
                                    mm(o_ps[:, poff:poff + pw],
                                       vv[:, pj * 128:(pj + 1) * 128],
                                       patt[:, 0:pw], pj == 0, False)
                                    mm(d_ps[:, poff:poff + pw], onc_sb[:],
                                       patt[:, 0:pw], pj == 0, False)
                                pend_att = (att, off, w, j)
                            patt, poff, pw, pj = pend_att
                            mm(o_ps[:, poff:poff + pw],
                               vv[:, pj * 128:(pj + 1) * 128],
                               patt[:, 0:pw], pj == 0, True)
                            mm(d_ps[:, poff:poff + pw], onc_sb[:],
                               patt[:, 0:pw], pj == 0, True)
                            pend_norm = (o_ps, d_ps, h, i)
                    flush_norm(pend_norm)

                # ---------------- phase C: output projection --------------------
                with tc.tile_pool(name=f"woC{_rep}", bufs=3) as wop, \
                     tc.tile_pool(name=f"ysb{_rep}", bufs=6) as ysb, \
                     tc.tile_pool(name=f"psY{_rep}", bufs=4, space="PSUM") as psY:
                    for d in range(4):
                        wo_sb = wop.tile([128, 4 * 512], mdt, name="wo_sb")
                        wor = wo_sb.rearrange("p (c n) -> p c n", c=4)
                        wos = wo_d.rearrange("(c p) n -> p c n", p=128)
                        for c in range(4):
                            nc.sync.dma_start(
                                wor[:, c, :],
                                wos[:, c, d * 512:(d + 1) * 512])
                        for t in range(TT):
                            y_ps = psY.tile([128, 512], f32, name="y_ps")
                            for lc in range(4):
                                mm(y_ps[:],
                                   oT[:, lc * T + t * 128: lc * T + (t + 1) * 128],
                                   wo_sb[:, lc * 512:(lc + 1) * 512],
                                   lc == 0, lc == 3)
                            y_sb = ysb.tile([128, 512], f32, name="y_sb")
                            if (d * TT + t) % 2 == 0:
                                nc.scalar.copy(y_sb[:], y_ps[:])
                            else:
                                nc.vector.tensor_copy(y_sb[:], y_ps[:])
                            nc.sync.dma_start(
                                y_d[t * 128:(t + 1) * 128, d * 512:(d + 1) * 512],
                                y_sb[:])

    nc.compile()
    return nc


_EVOD = None


def _perm():
    global _EVOD
    if _EVOD is None:
        _EVOD = np.concatenate([np.arange(0, HD, 2), np.arange(1, HD, 2)])
    return _EVOD


def prepare_inputs(x, wq, wk, wv, wo, q_norm_w, k_norm_w, freqs_cos, freqs_sin):
    """Host-side sharding + layout prep. Returns (in_maps, shared_freqs)."""
    x = np.asarray(x, np.float32)
    wq = np.asarray(wq, np.float32)
    wk = np.asarray(wk, np.float32)
    wv = np.asarray(wv, np.float32)
    wo = np.asarray(wo, np.float32)
    qw = np.asarray(q_norm_w, np.float32)
    kw = np.asarray(k_norm_w, np.float32)
    cos = np.asarray(freqs_cos, np.float32)
    sin = np.asarray(freqs_sin, np.float32)

    perm = _perm()
    shared = bool(np.allclose(qw, kw))
    unit_w = bool(np.allclose(qw, 1.0) and np.allclose(kw, 1.0))

    def freq4(w):
        we, wo_ = w[0::2], w[1::2]
        return np.concatenate(
            [cos * we[None, :], sin * wo_[None, :],
             sin * we[None, :], cos * wo_[None, :]], axis=1)

    f8 = freq4(qw) if shared else np.concatenate([freq4(qw), freq4(kw)], axis=1)
    f8 = np.ascontiguousarray(f8, np.float32)

    msk = np.where(np.arange(128)[None, :] >= np.arange(128)[:, None],
                   np.float32(0.0), np.float32(NEG)).astype(np.float32)
    eye = np.eye(128, dtype=np.float32)
    onc = np.ones((128, 1), np.float32)
    onr = np.ones((1, 128), np.float32)

    if MM_DTYPE == "bf16":
        import ml_dtypes
        mnp = ml_dtypes.bfloat16
    else:
        mnp = np.float32
    tnp = mnp if MM_DTYPE == "bf16" else np.float32

    xTs = [np.ascontiguousarray(x[b].T).astype(mnp) for b in range(B)]
    eye = eye.astype(tnp)
    onc = onc.astype(mnp)
    in_maps = []
    for c in range(NCORES):
        b, g = divmod(c, KV)
        wq_g = wq[:, g * GW:(g + 1) * GW].reshape(D, NH, HD)[:, :, perm] \
            .reshape(D, GW)
        wk_g = wk[:, g * HD:(g + 1) * HD][:, perm]
        wv_g = wv[:, g * HD:(g + 1) * HD]
        wkv_g = np.ascontiguousarray(
            np.concatenate([wk_g, wv_g], axis=1)).astype(mnp)
        wo_g = np.ascontiguousarray(wo[g * GW:(g + 1) * GW, :]).astype(mnp)
        in_maps.append(dict(
            xT=xTs[b], wq=np.ascontiguousarray(wq_g).astype(mnp),
            wkv=wkv_g, wo=wo_g, f8=f8, msk=msk, eye=eye, onc=onc, onr=onr))
    return in_maps, shared, unit_w


def get_program(shared_freqs: bool, repeat: int = 1, timing: bool = False,
                unit_w: bool = True):
    key = (shared_freqs, MM_DTYPE, repeat, timing, unit_w)
    if key not in _prog_cache:
        _prog_cache[key] = _build(shared_freqs, repeat, timing, unit_w)
    return _prog_cache[key]


def kernel(**inputs):
    from concourse.bass_utils import run_bass_kernel_spmd

    in_maps, shared, unit_w = prepare_inputs(**inputs)
    nc = get_program(shared, unit_w=unit_w)
    res = run_bass_kernel_spmd(nc, in_maps, list(range(NCORES)))
    out = np.empty((B, T, D), np.float32)
    for b in range(B):
        acc = res.results[b * KV + 0]["y"].astype(np.float32)
        for g in range(1, KV):
            acc = acc + res.results[b * KV + g]["y"]
        out[b] = acc
    return out



# revision 7
# speedup vs baseline: 1.2568x; 1.1802x over previous
"""Trainium2 Bass kernel for nn_Attention_26182120636812 (GQA attention block).

Sharding: 8 cores = 2 (batch) x 4 (KV groups). Each core computes, for its
batch element b and kv-group g: the 4 query heads + 1 kv head of group g,
full causal attention over T=2048, and the partial output projection
y_part = o_g @ wo[g*512:(g+1)*512, :]. The host sums the 4 partials per batch.

v2 design (vs the f32r baseline):
 - all matmuls in bf16 (measured ~1.25x the f32r rate on real HW; accuracy
   budget ~5e-3 vs the 2e-2 gate)
 - scores computed transposed (tk x tq) in [128,1024] 2-bank PSUM tiles so
   exp batches two 128-token chunks per ACT instruction (352-cycle ACT
   overhead amortized)
 - causal mask added on the PE as an extra matmul (eye @ msk) inside the
   score accumulation group -- no DVE mask traffic
 - softmax denominators: per-chunk DVE adds into an fp16 accumulator
   (2-byte dtypes keep the DVE in 4x mode; fp16 keeps accumulation error
   ~0.1% where bf16 would lose several %), then one ones-matmul per (h,i)
   for the cross-partition reduction, reciprocal on DVE, broadcast via a
   K=1 matmul
 - phase C (y = o @ wo) interleaved per query supertile into phase B's
   instruction stream so PE never idles while ACT runs exp
 - engine balance: ACT does the PSUM->SBUF qkv/oT copies + exp; DVE does
   rope (bf16 4x), denominators, normalization muls, y copies
"""
import sys

for _p in ("/opt/trn_rl_repo",):
    if _p not in sys.path:
        sys.path.insert(0, _p)

import numpy as np

B, T, D = 2, 2048, 2048
H, KV, HD = 16, 4, 128
NCORES = 8
NH = H // KV          # 4 q heads per core
GW = NH * HD          # 512: per-core q / o width
TT = T // 128         # 16 token tiles
NI = T // 512         # 4 query super-tiles
DC = D // 128         # 16 contraction chunks over D
EPS = 1e-6
SCALE = 1.0 / float(np.sqrt(HD))
NEG = -1.0e30

_prog_cache = {}


def _build(shared_freqs: bool, repeat: int = 1, timing: bool = False,
           unit_w: bool = True):
    import concourse.bacc as bacc
    import concourse.mybir as mybir
    import concourse.tile as tile

    dt = mybir.dt
    f32 = dt.float32
    bf16 = dt.bfloat16
    fp16 = dt.float16
    AF = mybir.ActivationFunctionType

    nc = bacc.Bacc("TRN2", target_bir_lowering=False, debug=False,
                   num_devices=NCORES)
    ikind = "Internal" if timing else "ExternalInput"
    okind = "Internal" if timing else "ExternalOutput"
    xT_d = nc.dram_tensor("xT", [D, T], bf16, kind=ikind).ap()
    wq_d = nc.dram_tensor("wq", [D, GW], bf16, kind=ikind).ap()
    wkv_d = nc.dram_tensor("wkv", [D, 2 * HD], bf16, kind=ikind).ap()
    wo_d = nc.dram_tensor("wo", [GW, D], bf16, kind=ikind).ap()
    nf = 256 if shared_freqs else 512
    f8_d = nc.dram_tensor("f8", [T, nf], bf16, kind=ikind).ap()
    msk_d = nc.dram_tensor("msk", [128, 128], bf16, kind=ikind).ap()
    eye_d = nc.dram_tensor("eye", [128, 128], bf16, kind=ikind).ap()
    onc_d = nc.dram_tensor("onc", [128, 1], fp16, kind=ikind).ap()
    onr_d = nc.dram_tensor("onr", [1, 128], dt.float32r, kind=ikind).ap()
    y_d = nc.dram_tensor("y", [T, D], bf16, kind=okind).ap()
    if timing:
        din = nc.dram_tensor("din", [128, 4], f32, kind="ExternalInput").ap()
        dout = nc.dram_tensor("dout", [128, 4], f32,
                              kind="ExternalOutput").ap()

    def mm(out, lhsT, rhs, start, stop):
        nc.tensor.matmul(out, lhsT, rhs, start=start, stop=stop,
                         skip_group_check=True)

    with nc.allow_low_precision(reason="bf16 matmuls, fp16 denom accum"), \
         tile.TileContext(nc) as tc:
        with tc.tile_pool(name="const", bufs=1) as cpool, \
             tc.tile_pool(name="resid", bufs=1) as rpool:
            if timing:
                dsb = cpool.tile([128, 4], f32)
                nc.sync.dma_start(dsb[:], din[:])
                nc.sync.dma_start(dout[:], dsb[:])
            msk_sb = cpool.tile([128, 128], bf16)
            nc.sync.dma_start(msk_sb[:], msk_d[:])
            eye_sb = cpool.tile([128, 128], bf16)
            nc.sync.dma_start(eye_sb[:], eye_d[:])
            onc_sb = cpool.tile([128, 1], fp16)
            nc.sync.dma_start(onc_sb[:], onc_d[:])
            onr_sb = cpool.tile([1, 128], dt.float32r)
            nc.sync.dma_start(onr_sb[:], onr_d[:])
            eps_sb = cpool.tile([128, 1], f32)
            nc.vector.memset(eps_sb[:], EPS)

            # residents: qT/kT head_dim-major for scores, qks holds the raw
            # q|k|v projections per ttile (v consumed in place by AV), oT
            # per head, wo for phase C
            qT = rpool.tile([128, NH * T], bf16)   # head h at [:, h*T:(h+1)*T]
            kT = rpool.tile([128, T], bf16)
            qks = rpool.tile([128, TT * 768], bf16)
            oT = rpool.tile([128, NH * T], bf16)
            wo_sb = rpool.tile([128, NH * D], bf16)  # lc chunk at [:,lc*D:...]

            for _rep in range(repeat):
                # ---------------- phase A: projections + rmsnorm + rope -----
                with tc.tile_pool(name=f"wA{_rep}", bufs=1) as wA, \
                     tc.tile_pool(name=f"xs{_rep}", bufs=2) as xsp, \
                     tc.tile_pool(name=f"fA{_rep}", bufs=2) as fap, \
                     tc.tile_pool(name=f"qrp{_rep}", bufs=2) as qrp, \
                     tc.tile_pool(name=f"smA{_rep}", bufs=2) as smp, \
                     tc.tile_pool(name=f"psA{_rep}", bufs=2,
                                  space="PSUM") as psA, \
                     tc.tile_pool(name=f"psT{_rep}", bufs=2,
                                  space="PSUM") as psT:
                    wq_sb = wA.tile([128, DC * GW], bf16)
                    wkv_sb = wA.tile([128, DC * 2 * HD], bf16)
                    wqr = wq_sb.rearrange("p (c n) -> p c n", c=DC)
                    wqs = wq_d.rearrange("(c p) n -> p c n", p=128)
                    wkr = wkv_sb.rearrange("p (c n) -> p c n", c=DC)
                    wks = wkv_d.rearrange("(c p) n -> p c n", p=128)
                    for c in range(DC):
                        nc.sync.dma_start(wqr[:, c, :], wqs[:, c, :])
                        nc.sync.dma_start(wkr[:, c, :], wks[:, c, :])
                    # wo resident for phase C; DMA overlaps phase A compute
                    wor = wo_sb.rearrange("p (c n) -> p c n", c=NH)
                    wos = wo_d.rearrange("(c p) n -> p c n", p=128)
                    for c in range(NH):
                        nc.sync.dma_start(wor[:, c, :], wos[:, c, :])
                    wq_v = wq_sb.rearrange("p (c n) -> p c n", c=DC)
                    wkv_v = wkv_sb.rearrange("p (c n) -> p c n", c=DC)

                    xs = None
                    pend_qr = None
                    for t in range(TT):
                        g2, half = divmod(t, 2)
                        if half == 0:
                            xs = xsp.tile([128, DC * 256], bf16, name="xs")
                            xsr = xs.rearrange("p (c n) -> p c n", c=DC)
                            xss = xT_d.rearrange("(c p) n -> p c n", p=128)
                            for c in range(DC):
                                nc.sync.dma_start(
                                    xsr[:, c, :],
                                    xss[:, c, g2 * 256:(g2 + 1) * 256])
                        xs_v = xs.rearrange("p (c n) -> p c n", c=DC)
                        f8t = fap.tile([128, nf], bf16, name="f8t")
                        nc.sync.dma_start(f8t[:], f8_d[t * 128:(t + 1) * 128, :])

                        # q at [0:512], k at [512:640], v at [640:768]
                        qkv_ps = psA.tile([128, 768], f32, name="qkv_ps")
                        for c in range(DC):
                            xsl = xs_v[:, c, half * 128:(half + 1) * 128]
                            mm(qkv_ps[:, 0:GW], xsl, wq_v[:, c, :],
                               c == 0, c == DC - 1)
                            mm(qkv_ps[:, GW:GW + 2 * HD], xsl, wkv_v[:, c, :],
                               c == 0, c == DC - 1)

                        ssq = smp.tile([128, 8], f32, name="ssq")
                        if not unit_w:
                            # rmsnorm sums-of-squares from the raw projections
                            sqs = smp.tile([128, 128], f32, name="sqs")
                            for h5 in range(5):
                                nc.scalar.activation(
                                    sqs[:], qkv_ps[:, h5 * 128:(h5 + 1) * 128],
                                    AF.Square, accum_out=ssq[:, h5:h5 + 1])

                        # single wide PSUM->SBUF copy (ACT); v lands resident
                        qks_t = qks[:, t * 768:(t + 1) * 768]
                        nc.scalar.copy(qks_t, qkv_ps[:])

                        # rope on DVE in bf16 (4x mode); freqs carry the
                        # rmsnorm weights; [evens|odds] within each 128 block
                        qr = qrp.tile([128, 640], bf16, name="qr")
                        t1 = qrp.tile([128, 256], bf16, name="t1")
                        t2 = qrp.tile([128, 256], bf16, name="t2")
                        q_v = qks_t[:, 0:GW].rearrange("p (h x) -> p h x", h=4)
                        qe, qo = q_v[:, :, 0:64], q_v[:, :, 64:128]
                        t1_v = t1.rearrange("p (h x) -> p h x", h=4)
                        t2_v = t2.rearrange("p (h x) -> p h x", h=4)
                        qr_v = qr.rearrange("p (h x) -> p h x", h=5)

                        def fq(k4):  # freq slice broadcast over the 4 q heads
                            s = f8t[:, k4 * 64:(k4 + 1) * 64]
                            return s.rearrange("p (o x) -> p o x", o=1) \
                                    .broadcast_to([128, 4, 64])

                        nc.vector.tensor_mul(t1_v, qe, fq(0))          # e*cosE
                        nc.vector.tensor_mul(t2_v, qo, fq(1))          # o*sinO
                        nc.vector.tensor_sub(qr_v[:, 0:4, 0:64], t1_v, t2_v)
                        nc.vector.tensor_mul(t1_v, qe, fq(2))          # e*sinE
                        nc.vector.tensor_mul(t2_v, qo, fq(3))          # o*cosO
                        nc.vector.tensor_add(qr_v[:, 0:4, 64:128], t1_v, t2_v)

                        kf0 = 0 if shared_freqs else 4
                        ke = qks_t[:, GW:GW + 64]
                        ko = qks_t[:, GW + 64:GW + 128]
                        kt1 = smp.tile([128, 64], bf16, name="kt1")
                        kt2 = smp.tile([128, 64], bf16, name="kt2")

                        def fk(k4):
                            return f8t[:, (kf0 + k4) * 64:(kf0 + k4 + 1) * 64]

                        nc.vector.tensor_mul(kt1[:], ke, fk(0))
                        nc.vector.tensor_mul(kt2[:], ko, fk(1))
                        nc.vector.tensor_sub(qr[:, 512:576], kt1[:], kt2[:])
                        nc.vector.tensor_mul(kt1[:], ke, fk(2))
                        nc.vector.tensor_mul(kt2[:], ko, fk(3))
                        nc.vector.tensor_add(qr[:, 576:640], kt1[:], kt2[:])

                        if unit_w:
                            # rope is a pure rotation: take sums of squares
                            # from the rope output on DVE (bf16 4x)
                            sqs = smp.tile([128, 128], bf16, name="sqs")
                            for h5 in range(5):
                                sl = qr[:, h5 * 128:(h5 + 1) * 128]
                                nc.vector.scalar_tensor_tensor(
                                    sqs[:], sl, 1.0, sl,
                                    mybir.AluOpType.mult,
                                    mybir.AluOpType.mult,
                                    accum_out=ssq[:, h5:h5 + 1])
                        rstd = smp.tile([128, 8], f32, name="rstd")
                        nc.scalar.activation(rstd[:, 0:5], ssq[:, 0:5], AF.Sqrt,
                                             bias=eps_sb[:], scale=1.0 / HD)
                        rms = smp.tile([128, 8], f32, name="rms")
                        nc.vector.reciprocal(rms[:, 0:5], rstd[:, 0:5])
                        for h5 in range(5):
                            sl = qr[:, h5 * 128:(h5 + 1) * 128]
                            nc.vector.tensor_scalar_mul(sl, sl, rms[:, h5:h5 + 1])

                        # transpose each head block into the resident qT /
                        # kT -- deferred one ttile so the PE never waits on
                        # the DVE rope chain (software pipeline).
                        if pend_qr is not None:
                            pqr, pt = pend_qr
                            for h5 in range(5):
                                tp_ps = psT.tile([128, 128], bf16, name="tp_ps")
                                nc.tensor.transpose(
                                    tp_ps[:], pqr[:, h5 * 128:(h5 + 1) * 128],
                                    eye_sb[:])
                                dst = (qT[:, h5 * T + pt * 128:
                                          h5 * T + (pt + 1) * 128]
                                       if h5 < 4
                                       else kT[:, pt * 128:(pt + 1) * 128])
                                nc.scalar.copy(dst, tp_ps[:])
                        pend_qr = (qr, t)

                    pqr, pt = pend_qr
                    for h5 in range(5):
                        tp_ps = psT.tile([128, 128], bf16, name="tp_ps")
                        nc.tensor.transpose(
                            tp_ps[:], pqr[:, h5 * 128:(h5 + 1) * 128],
                            eye_sb[:])
                        dst = (qT[:, h5 * T + pt * 128:
                                  h5 * T + (pt + 1) * 128]
                               if h5 < 4 else kT[:, pt * 128:(pt + 1) * 128])
                        nc.scalar.copy(dst, tp_ps[:])

                # ---------------- phase B+C: attention + output proj --------
                with tc.tile_pool(name=f"attp{_rep}", bufs=3) as attp, \
                     tc.tile_pool(name=f"smB{_rep}", bufs=3) as smB, \
                     tc.tile_pool(name=f"ysb{_rep}", bufs=4) as ysb, \
                     tc.tile_pool(name=f"psS{_rep}", bufs=2,
                                  space="PSUM") as psS, \
                     tc.tile_pool(name=f"psO{_rep}", bufs=2,
                                  space="PSUM") as psO, \
                     tc.tile_pool(name=f"psY{_rep}", bufs=2,
                                  space="PSUM") as psY:
                    wo_v = wo_sb.rearrange("p (c n) -> p c n", c=NH)

                    def flush_norm(pn):
                        # PSUM slot reuse: d_ps borrows the y_ps slot, bc_ps
                        # borrows po's o_ps slot (po is fully read by the
                        # copy emitted before the bc matmul)
                        po, pdacc, ph, pi = pn
                        osl = oT[:, ph * T + pi * 512:ph * T + (pi + 1) * 512]
                        nc.vector.tensor_copy(osl, po[:])
                        d_ps = psY.tile([1, 512], f32, name="y_ps")
                        mm(d_ps[:], onc_sb[:], pdacc[:], True, True)
                        rec = smB.tile([1, 512], dt.float32r, name="rec")
                        nc.vector.reciprocal(rec[:], d_ps[:])
                        bc_ps = psO.tile([128, 512], f32, name="o_ps")
                        mm(bc_ps[:], onr_sb[:], rec[:], True, True)
                        nc.vector.tensor_mul(osl, osl, bc_ps[:])

                    def emit_y(pi, sub):
                        # one y tile of supertile pi: ttile t, dblock dblk
                        t = pi * 4 + sub // 4
                        dblk = sub % 4
                        y_ps = psY.tile([128, 512], f32, name="y_ps")
                        for lc in range(NH):
                            mm(y_ps[:],
                               oT[:, lc * T + t * 128:lc * T + (t + 1) * 128],
                               wo_v[:, lc, dblk * 512:(dblk + 1) * 512],
                               lc == 0, lc == NH - 1)
                        y_sb = ysb.tile([128, 512], bf16, name="y_sb")
                        if sub % 4 == 3:
                            nc.scalar.copy(y_sb[:], y_ps[:])
                        else:
                            nc.vector.tensor_copy(y_sb[:], y_ps[:])
                        nc.sync.dma_start(
                            y_d[t * 128:(t + 1) * 128,
                                dblk * 512:(dblk + 1) * 512], y_sb[:])

                    pend_norm = None
                    for i in range(NI):
                        for h in range(NH):
                            o_ps = psO.tile([128, 512], f32, name="o_ps")
                            dacc = smB.tile([128, 512], fp16, name="dacc")
                            npair = 2 * i + 2
                            pend_att = None
                            first_chunk = True
                            for jp in range(npair):
                                s2 = psS.tile([128, 1024], f32, name="s2")
                                offs = []
                                for half in range(2):
                                    j = 2 * jp + half
                                    r = j - 4 * i
                                    off = 128 * r if r >= 0 else 0
                                    w = 512 - off
                                    offs.append((j, off, w))
                                    slot = half * 512
                                    mm(s2[:, slot + off:slot + off + w],
                                       kT[:, j * 128:(j + 1) * 128],
                                       qT[:, h * T + i * 512 + off:
                                           h * T + i * 512 + off + w],
                                       True, r < 0)
                                    if r >= 0:
                                        # causal triangle via PE: += eye @ msk
                                        mm(s2[:, slot + off:slot + off + 128],
                                           eye_sb[:], msk_sb[:], False, True)
                                # batched exp over both chunks (garbage in
                                # the gap columns is never read downstream)
                                att2 = attp.tile([128, 1024], bf16, name="att2")
                                a0 = offs[0][1]
                                nc.scalar.activation(att2[:, a0:1024],
                                                     s2[:, a0:1024],
                                                     AF.Exp, scale=SCALE)
                                # previous pair's AV + denominator adds run
                                # inside this pair's stream (software pipe)
                                if jp == 1 and pend_norm is not None:
                                    flush_norm(pend_norm)
                                    pend_norm = None
                                if pend_att is not None:
                                    patt, poffs = pend_att
                                    for half in range(2):
                                        pj, poff, pw = poffs[half]
                                        slot = half * 512
                                        mm(o_ps[:, poff:poff + pw],
                                           qks[:, pj * 768 + 640:
                                               (pj + 1) * 768],
                                           patt[:, slot + poff:
                                                slot + poff + pw],
                                           pj == 0, False)
                                        asl = patt[:, slot + poff:
                                                   slot + poff + pw]
                                        dsl = dacc[:, poff:poff + pw]
                                        if first_chunk:
                                            nc.vector.tensor_copy(dsl, asl)
                                            first_chunk = False
                                        else:
                                            nc.vector.tensor_add(dsl, dsl, asl)
                                pend_att = (att2, offs)
                            patt, poffs = pend_att
                            for half in range(2):
                                pj, poff, pw = poffs[half]
                                slot = half * 512
                                mm(o_ps[:, poff:poff + pw],
                                   qks[:, pj * 768 + 640:(pj + 1) * 768],
                                   patt[:, slot + poff:slot + poff + pw],
                                   pj == 0, half == 1)
                                asl = patt[:, slot + poff:slot + poff + pw]
                                dsl = dacc[:, poff:poff + pw]
                                if first_chunk:
                                    nc.vector.tensor_copy(dsl, asl)
                                    first_chunk = False
                                else:
                                    nc.vector.tensor_add(dsl, dsl, asl)
                            pend_norm = (o_ps, dacc, h, i)
                            # interleave previous supertile's output proj
                            if i > 0:
                                for sub in range(4 * h, 4 * h + 4):
                                    emit_y(i - 1, sub)
                    flush_norm(pend_norm)
                    for sub in range(16):
                        emit_y(NI - 1, sub)

    nc.compile()
    return nc


_EVOD = None


def _perm():
    global _EVOD
    if _EVOD is None:
        _EVOD = np.concatenate([np.arange(0, HD, 2), np.arange(1, HD, 2)])
    return _EVOD


def prepare_inputs(x, wq, wk, wv, wo, q_norm_w, k_norm_w, freqs_cos, freqs_sin):
    """Host-side sharding + layout prep. Returns (in_maps, shared, unit_w)."""
    import ml_dtypes
    bnp = ml_dtypes.bfloat16

    x = np.asarray(x, np.float32)
    wq = np.asarray(wq, np.float32)
    wk = np.asarray(wk, np.float32)
    wv = np.asarray(wv, np.float32)
    wo = np.asarray(wo, np.float32)
    qw = np.asarray(q_norm_w, np.float32)
    kw = np.asarray(k_norm_w, np.float32)
    cos = np.asarray(freqs_cos, np.float32)
    sin = np.asarray(freqs_sin, np.float32)

    perm = _perm()
    shared = bool(np.allclose(qw, kw))
    unit_w = bool(np.allclose(qw, 1.0) and np.allclose(kw, 1.0))

    def freq4(w):
        we, wo_ = w[0::2], w[1::2]
        return np.concatenate(
            [cos * we[None, :], sin * wo_[None, :],
             sin * we[None, :], cos * wo_[None, :]], axis=1)

    f8 = freq4(qw) if shared else np.concatenate([freq4(qw), freq4(kw)], axis=1)
    f8 = np.ascontiguousarray(f8).astype(bnp)

    msk = np.where(np.arange(128)[None, :] >= np.arange(128)[:, None],
                   np.float32(0.0), np.float32(NEG)).astype(bnp)
    eye = np.eye(128, dtype=np.float32).astype(bnp)
    onc = np.ones((128, 1), np.float16)
    onr = np.ones((1, 128), np.float32)

    xTs = [np.ascontiguousarray(x[b].T).astype(bnp) for b in range(B)]
    in_maps = []
    for c in range(NCORES):
        b, g = divmod(c, KV)
        wq_g = wq[:, g * GW:(g + 1) * GW].reshape(D, NH, HD)[:, :, perm] \
            .reshape(D, GW)
        wk_g = wk[:, g * HD:(g + 1) * HD][:, perm]
        wv_g = wv[:, g * HD:(g + 1) * HD]
        wkv_g = np.ascontiguousarray(
            np.concatenate([wk_g, wv_g], axis=1)).astype(bnp)
        wo_g = np.ascontiguousarray(wo[g * GW:(g + 1) * GW, :]).astype(bnp)
        in_maps.append(dict(
            xT=xTs[b], wq=np.ascontiguousarray(wq_g).astype(bnp),
            wkv=wkv_g, wo=wo_g, f8=f8, msk=msk, eye=eye, onc=onc, onr=onr))
    return in_maps, shared, unit_w


def get_program(shared_freqs: bool, repeat: int = 1, timing: bool = False,
                unit_w: bool = True):
    key = (shared_freqs, repeat, timing, unit_w)
    if key not in _prog_cache:
        _prog_cache[key] = _build(shared_freqs, repeat, timing, unit_w)
    return _prog_cache[key]


def kernel(**inputs):
    from concourse.bass_utils import run_bass_kernel_spmd

    in_maps, shared, unit_w = prepare_inputs(**inputs)
    nc = get_program(shared, unit_w=unit_w)
    res = run_bass_kernel_spmd(nc, in_maps, list(range(NCORES)))
    out = np.empty((B, T, D), np.float32)
    for b in range(B):
        acc = res.results[b * KV + 0]["y"].astype(np.float32)
        for g in range(1, KV):
            acc = acc + res.results[b * KV + g]["y"].astype(np.float32)
        out[b] = acc
    return out


# revision 9
# speedup vs baseline: 1.2729x; 1.0128x over previous
"""Trainium2 Bass kernel for nn_Attention_26182120636812 (GQA attention block).

Sharding: 8 cores = 2 (batch) x 4 (KV groups). Each core computes, for its
batch element b and kv-group g: the 4 query heads + 1 kv head of group g,
full causal attention over T=2048, and the partial output projection
y_part = o_g @ wo[g*512:(g+1)*512, :]. The host sums the 4 partials per batch.

v2 design (vs the f32r baseline):
 - all matmuls in bf16 (measured ~1.25x the f32r rate on real HW; accuracy
   budget ~5e-3 vs the 2e-2 gate)
 - scores computed transposed (tk x tq) in [128,1024] 2-bank PSUM tiles so
   exp batches two 128-token chunks per ACT instruction (352-cycle ACT
   overhead amortized)
 - causal mask added on the PE as an extra matmul (eye @ msk) inside the
   score accumulation group -- no DVE mask traffic
 - softmax denominators: per-chunk DVE adds into an fp16 accumulator
   (2-byte dtypes keep the DVE in 4x mode; fp16 keeps accumulation error
   ~0.1% where bf16 would lose several %), then one ones-matmul per (h,i)
   for the cross-partition reduction, reciprocal on DVE, broadcast via a
   K=1 matmul
 - phase C (y = o @ wo) interleaved per query supertile into phase B's
   instruction stream so PE never idles while ACT runs exp
 - engine balance: ACT does the PSUM->SBUF qkv/oT copies + exp; DVE does
   rope (bf16 4x), denominators, normalization muls, y copies
"""
import sys

for _p in ("/opt/trn_rl_repo",):
    if _p not in sys.path:
        sys.path.insert(0, _p)

import numpy as np

B, T, D = 2, 2048, 2048
H, KV, HD = 16, 4, 128
NCORES = 8
NH = H // KV          # 4 q heads per core
GW = NH * HD          # 512: per-core q / o width
TT = T // 128         # 16 token tiles
NI = T // 512         # 4 query super-tiles
DC = D // 128         # 16 contraction chunks over D
EPS = 1e-6
SCALE = 1.0 / float(np.sqrt(HD))
NEG = -1.0e30

_prog_cache = {}


def _build(shared_freqs: bool, repeat: int = 1, timing: bool = False,
           unit_w: bool = True, parts: str = "ABC"):
    import concourse.bacc as bacc
    import concourse.mybir as mybir
    import concourse.tile as tile

    dt = mybir.dt
    f32 = dt.float32
    bf16 = dt.bfloat16
    fp16 = dt.float16
    AF = mybir.ActivationFunctionType

    nc = bacc.Bacc("TRN2", target_bir_lowering=False, debug=False,
                   num_devices=NCORES)
    ikind = "Internal" if timing else "ExternalInput"
    okind = "Internal" if timing else "ExternalOutput"
    xT_d = nc.dram_tensor("xT", [D, T], bf16, kind=ikind).ap()
    wq_d = nc.dram_tensor("wq", [D, GW], bf16, kind=ikind).ap()
    wkv_d = nc.dram_tensor("wkv", [D, 2 * HD], bf16, kind=ikind).ap()
    wo_d = nc.dram_tensor("wo", [GW, D], bf16, kind=ikind).ap()
    nf = 256 if shared_freqs else 512
    f8_d = nc.dram_tensor("f8", [T, nf], bf16, kind=ikind).ap()
    msk_d = nc.dram_tensor("msk", [128, 128], bf16, kind=ikind).ap()
    eye_d = nc.dram_tensor("eye", [128, 128], bf16, kind=ikind).ap()
    onc_d = nc.dram_tensor("onc", [128, 1], fp16, kind=ikind).ap()
    onr_d = nc.dram_tensor("onr", [1, 128], dt.float32r, kind=ikind).ap()
    y_d = nc.dram_tensor("y", [T, D], bf16, kind=okind).ap()
    if timing:
        din = nc.dram_tensor("din", [128, 4], f32, kind="ExternalInput").ap()
        dout = nc.dram_tensor("dout", [128, 4], f32,
                              kind="ExternalOutput").ap()

    def mm(out, lhsT, rhs, start, stop):
        nc.tensor.matmul(out, lhsT, rhs, start=start, stop=stop,
                         skip_group_check=True)

    with nc.allow_low_precision(reason="bf16 matmuls, fp16 denom accum"), \
         tile.TileContext(nc) as tc:
        with tc.tile_pool(name="const", bufs=1) as cpool, \
             tc.tile_pool(name="resid", bufs=1) as rpool:
            if timing:
                dsb = cpool.tile([128, 4], f32)
                nc.sync.dma_start(dsb[:], din[:])
                nc.sync.dma_start(dout[:], dsb[:])
            msk_sb = cpool.tile([128, 128], bf16)
            nc.sync.dma_start(msk_sb[:], msk_d[:])
            eye_sb = cpool.tile([128, 128], bf16)
            nc.sync.dma_start(eye_sb[:], eye_d[:])
            onc_sb = cpool.tile([128, 1], fp16)
            nc.sync.dma_start(onc_sb[:], onc_d[:])
            onr_sb = cpool.tile([1, 128], dt.float32r)
            nc.sync.dma_start(onr_sb[:], onr_d[:])
            eps_sb = cpool.tile([128, 1], f32)
            nc.vector.memset(eps_sb[:], EPS)

            # residents: qT/kT head_dim-major for scores, qks holds the raw
            # q|k|v projections per ttile (v consumed in place by AV), oT
            # per head, wo for phase C
            qT = rpool.tile([128, NH * T], bf16)   # head h at [:, h*T:(h+1)*T]
            kT = rpool.tile([128, T], bf16)
            qks = rpool.tile([128, TT * 768], bf16)
            oT = rpool.tile([128, NH * T], bf16)
            wo_sb = rpool.tile([128, NH * D], bf16)  # lc chunk at [:,lc*D:...]

            for _rep in range(repeat):
                # ---------------- phase A: projections + rmsnorm + rope -----
                if "A" not in parts:
                    break
                with tc.tile_pool(name=f"wA{_rep}", bufs=1) as wA, \
                     tc.tile_pool(name=f"xs{_rep}", bufs=2) as xsp, \
                     tc.tile_pool(name=f"fA{_rep}", bufs=2) as fap, \
                     tc.tile_pool(name=f"qrp{_rep}", bufs=2) as qrp, \
                     tc.tile_pool(name=f"smA{_rep}", bufs=2) as smp, \
                     tc.tile_pool(name=f"psA{_rep}", bufs=2,
                                  space="PSUM") as psA, \
                     tc.tile_pool(name=f"psT{_rep}", bufs=2,
                                  space="PSUM") as psT:
                    wq_sb = wA.tile([128, DC * GW], bf16)
                    wkv_sb = wA.tile([128, DC * 2 * HD], bf16)
                    wqr = wq_sb.rearrange("p (c n) -> p c n", c=DC)
                    wqs = wq_d.rearrange("(c p) n -> p c n", p=128)
                    wkr = wkv_sb.rearrange("p (c n) -> p c n", c=DC)
                    wks = wkv_d.rearrange("(c p) n -> p c n", p=128)
                    for c in range(DC):
                        nc.sync.dma_start(wqr[:, c, :], wqs[:, c, :])
                        nc.sync.dma_start(wkr[:, c, :], wks[:, c, :])
                    # wo resident for phase C; DMA overlaps phase A compute
                    wor = wo_sb.rearrange("p (c n) -> p c n", c=NH)
                    wos = wo_d.rearrange("(c p) n -> p c n", p=128)
                    for c in range(NH):
                        nc.sync.dma_start(wor[:, c, :], wos[:, c, :])
                    wq_v = wq_sb.rearrange("p (c n) -> p c n", c=DC)
                    wkv_v = wkv_sb.rearrange("p (c n) -> p c n", c=DC)

                    xs = None
                    pend_qr = None
                    for t in range(TT):
                        g2, half = divmod(t, 2)
                        if half == 0:
                            xs = xsp.tile([128, DC * 256], bf16, name="xs")
                            xsr = xs.rearrange("p (c n) -> p c n", c=DC)
                            xss = xT_d.rearrange("(c p) n -> p c n", p=128)
                            for c in range(DC):
                                nc.sync.dma_start(
                                    xsr[:, c, :],
                                    xss[:, c, g2 * 256:(g2 + 1) * 256])
                        xs_v = xs.rearrange("p (c n) -> p c n", c=DC)
                        f8t = fap.tile([128, nf], bf16, name="f8t")
                        nc.sync.dma_start(f8t[:], f8_d[t * 128:(t + 1) * 128, :])

                        # q at [0:512], k at [512:640], v at [640:768]
                        qkv_ps = psA.tile([128, 768], f32, name="qkv_ps")
                        for c in range(DC):
                            xsl = xs_v[:, c, half * 128:(half + 1) * 128]
                            mm(qkv_ps[:, 0:GW], xsl, wq_v[:, c, :],
                               c == 0, c == DC - 1)
                            mm(qkv_ps[:, GW:GW + 2 * HD], xsl, wkv_v[:, c, :],
                               c == 0, c == DC - 1)

                        ssq = smp.tile([128, 8], f32, name="ssq")
                        if not unit_w:
                            # rmsnorm sums-of-squares from the raw projections
                            sqs = smp.tile([128, 128], f32, name="sqs")
                            for h5 in range(5):
                                nc.scalar.activation(
                                    sqs[:], qkv_ps[:, h5 * 128:(h5 + 1) * 128],
                                    AF.Square, accum_out=ssq[:, h5:h5 + 1])

                        # single wide PSUM->SBUF copy (ACT); v lands resident
                        qks_t = qks[:, t * 768:(t + 1) * 768]
                        nc.scalar.copy(qks_t, qkv_ps[:])

                        # rope on DVE in bf16 (4x mode); freqs carry the
                        # rmsnorm weights; [evens|odds] within each 128 block
                        qr = qrp.tile([128, 640], bf16, name="qr")
                        t1 = qrp.tile([128, 256], bf16, name="t1")
                        t2 = qrp.tile([128, 256], bf16, name="t2")
                        q_v = qks_t[:, 0:GW].rearrange("p (h x) -> p h x", h=4)
                        qe, qo = q_v[:, :, 0:64], q_v[:, :, 64:128]
                        t1_v = t1.rearrange("p (h x) -> p h x", h=4)
                        t2_v = t2.rearrange("p (h x) -> p h x", h=4)
                        qr_v = qr.rearrange("p (h x) -> p h x", h=5)

                        def fq(k4):  # freq slice broadcast over the 4 q heads
                            s = f8t[:, k4 * 64:(k4 + 1) * 64]
                            return s.rearrange("p (o x) -> p o x", o=1) \
                                    .broadcast_to([128, 4, 64])

                        nc.vector.tensor_mul(t1_v, qe, fq(0))          # e*cosE
                        nc.vector.tensor_mul(t2_v, qo, fq(1))          # o*sinO
                        nc.vector.tensor_sub(qr_v[:, 0:4, 0:64], t1_v, t2_v)
                        nc.vector.tensor_mul(t1_v, qe, fq(2))          # e*sinE
                        nc.vector.tensor_mul(t2_v, qo, fq(3))          # o*cosO
                        nc.vector.tensor_add(qr_v[:, 0:4, 64:128], t1_v, t2_v)

                        kf0 = 0 if shared_freqs else 4
                        ke = qks_t[:, GW:GW + 64]
                        ko = qks_t[:, GW + 64:GW + 128]
                        kt1 = smp.tile([128, 64], bf16, name="kt1")
                        kt2 = smp.tile([128, 64], bf16, name="kt2")

                        def fk(k4):
                            return f8t[:, (kf0 + k4) * 64:(kf0 + k4 + 1) * 64]

                        nc.vector.tensor_mul(kt1[:], ke, fk(0))
                        nc.vector.tensor_mul(kt2[:], ko, fk(1))
                        nc.vector.tensor_sub(qr[:, 512:576], kt1[:], kt2[:])
                        nc.vector.tensor_mul(kt1[:], ke, fk(2))
                        nc.vector.tensor_mul(kt2[:], ko, fk(3))
                        nc.vector.tensor_add(qr[:, 576:640], kt1[:], kt2[:])

                        if unit_w:
                            # rope is a pure rotation: take sums of squares
                            # from the rope output on DVE (bf16 4x)
                            sqs = smp.tile([128, 128], bf16, name="sqs")
                            for h5 in range(5):
                                sl = qr[:, h5 * 128:(h5 + 1) * 128]
                                nc.vector.scalar_tensor_tensor(
                                    sqs[:], sl, 1.0, sl,
                                    mybir.AluOpType.mult,
                                    mybir.AluOpType.mult,
                                    accum_out=ssq[:, h5:h5 + 1])
                        rstd = smp.tile([128, 8], f32, name="rstd")
                        nc.scalar.activation(rstd[:, 0:5], ssq[:, 0:5], AF.Sqrt,
                                             bias=eps_sb[:], scale=1.0 / HD)
                        rms = smp.tile([128, 8], f32, name="rms")
                        nc.vector.reciprocal(rms[:, 0:5], rstd[:, 0:5])
                        for h5 in range(5):
                            sl = qr[:, h5 * 128:(h5 + 1) * 128]
                            nc.vector.tensor_scalar_mul(sl, sl, rms[:, h5:h5 + 1])

                        # transpose each head block into the resident qT /
                        # kT -- deferred one ttile so the PE never waits on
                        # the DVE rope chain (software pipeline).
                        if pend_qr is not None:
                            pqr, pt = pend_qr
                            for h5 in range(5):
                                tp_ps = psT.tile([128, 128], bf16, name="tp_ps")
                                nc.tensor.transpose(
                                    tp_ps[:], pqr[:, h5 * 128:(h5 + 1) * 128],
                                    eye_sb[:])
                                dst = (qT[:, h5 * T + pt * 128:
                                          h5 * T + (pt + 1) * 128]
                                       if h5 < 4
                                       else kT[:, pt * 128:(pt + 1) * 128])
                                nc.scalar.copy(dst, tp_ps[:])
                        pend_qr = (qr, t)

                    pqr, pt = pend_qr
                    for h5 in range(5):
                        tp_ps = psT.tile([128, 128], bf16, name="tp_ps")
                        nc.tensor.transpose(
                            tp_ps[:], pqr[:, h5 * 128:(h5 + 1) * 128],
                            eye_sb[:])
                        dst = (qT[:, h5 * T + pt * 128:
                                  h5 * T + (pt + 1) * 128]
                               if h5 < 4 else kT[:, pt * 128:(pt + 1) * 128])
                        nc.scalar.copy(dst, tp_ps[:])

                # ---------------- phase B+C: attention + output proj --------
                if "B" not in parts:
                    continue
                with tc.tile_pool(name=f"attp{_rep}", bufs=3) as attp, \
                     tc.tile_pool(name=f"smB{_rep}", bufs=3) as smB, \
                     tc.tile_pool(name=f"ysb{_rep}", bufs=4) as ysb, \
                     tc.tile_pool(name=f"psS{_rep}", bufs=2,
                                  space="PSUM") as psS, \
                     tc.tile_pool(name=f"psO{_rep}", bufs=2,
                                  space="PSUM") as psO, \
                     tc.tile_pool(name=f"psY{_rep}", bufs=2,
                                  space="PSUM") as psY:
                    wo_v = wo_sb.rearrange("p (c n) -> p c n", c=NH)

                    def flush_norm(pn):
                        # PSUM slot reuse: d_ps borrows the y_ps slot, bc_ps
                        # borrows po's o_ps slot (po is fully read by the
                        # copy emitted before the bc matmul)
                        po, pdacc, ph, pi = pn
                        osl = oT[:, ph * T + pi * 512:ph * T + (pi + 1) * 512]
                        nc.vector.tensor_copy(osl, po[:])
                        d_ps = psY.tile([1, 512], f32, name="y_ps")
                        mm(d_ps[:], onc_sb[:], pdacc[:], True, True)
                        rec = smB.tile([1, 512], dt.float32r, name="rec")
                        nc.vector.reciprocal(rec[:], d_ps[:])
                        bc_ps = psO.tile([128, 512], f32, name="o_ps")
                        mm(bc_ps[:], onr_sb[:], rec[:], True, True)
                        nc.vector.tensor_mul(osl, osl, bc_ps[:])

                    def emit_y(pi, sub):
                        # one y tile of supertile pi: ttile t, dblock dblk
                        t = pi * 4 + sub // 4
                        dblk = sub % 4
                        y_ps = psY.tile([128, 512], f32, name="y_ps")
                        for lc in range(NH):
                            mm(y_ps[:],
                               oT[:, lc * T + t * 128:lc * T + (t + 1) * 128],
                               wo_v[:, lc, dblk * 512:(dblk + 1) * 512],
                               lc == 0, lc == NH - 1)
                        y_sb = ysb.tile([128, 512], bf16, name="y_sb")
                        nc.vector.tensor_copy(y_sb[:], y_ps[:])
                        nc.sync.dma_start(
                            y_d[t * 128:(t + 1) * 128,
                                dblk * 512:(dblk + 1) * 512], y_sb[:])

                    pend_norm = None
                    ysub = 0   # next y tile of supertile i-1 to emit
                    for i in range(NI):
                        ysub = 0
                        for h in range(NH):
                            o_ps = psO.tile([128, 512], f32, name="o_ps")
                            dacc = smB.tile([128, 512], fp16, name="dacc")
                            npair = 2 * i + 2
                            pend_att = None
                            first_chunk = True
                            for jp in range(npair):
                                if i > 0 and "C" in parts and ysub < 16 \
                                        and (jp > 0 or h > 0):
                                    emit_y(i - 1, ysub)
                                    ysub += 1
                                s2 = psS.tile([128, 1024], f32, name="s2")
                                offs = []
                                for half in range(2):
                                    j = 2 * jp + half
                                    r = j - 4 * i
                                    off = 128 * r if r >= 0 else 0
                                    w = 512 - off
                                    offs.append((j, off, w))
                                    slot = half * 512
                                    mm(s2[:, slot + off:slot + off + w],
                                       kT[:, j * 128:(j + 1) * 128],
                                       qT[:, h * T + i * 512 + off:
                                           h * T + i * 512 + off + w],
                                       True, r < 0)
                                    if r >= 0:
                                        # causal triangle via PE: += eye @ msk
                                        mm(s2[:, slot + off:slot + off + 128],
                                           eye_sb[:], msk_sb[:], False, True)
                                # batched exp over both chunks (garbage in
                                # the gap columns is never read downstream)
                                att2 = attp.tile([128, 1024], bf16, name="att2")
                                a0 = offs[0][1]
                                nc.scalar.activation(att2[:, a0:1024],
                                                     s2[:, a0:1024],
                                                     AF.Exp, scale=SCALE)
                                # previous pair's AV + denominator adds run
                                # inside this pair's stream (software pipe)
                                if jp == 1 and pend_norm is not None:
                                    flush_norm(pend_norm)
                                    pend_norm = None
                                if pend_att is not None:
                                    patt, poffs = pend_att
                                    for half in range(2):
                                        pj, poff, pw = poffs[half]
                                        slot = half * 512
                                        mm(o_ps[:, poff:poff + pw],
                                           qks[:, pj * 768 + 640:
                                               (pj + 1) * 768],
                                           patt[:, slot + poff:
                                                slot + poff + pw],
                                           pj == 0, False)
                                        asl = patt[:, slot + poff:
                                                   slot + poff + pw]
                                        dsl = dacc[:, poff:poff + pw]
                                        if first_chunk:
                                            nc.vector.tensor_copy(dsl, asl)
                                            first_chunk = False
                                        else:
                                            nc.vector.tensor_add(dsl, dsl, asl)
                                pend_att = (att2, offs)
                            patt, poffs = pend_att
                            for half in range(2):
                                pj, poff, pw = poffs[half]
                                slot = half * 512
                                mm(o_ps[:, poff:poff + pw],
                                   qks[:, pj * 768 + 640:(pj + 1) * 768],
                                   patt[:, slot + poff:slot + poff + pw],
                                   pj == 0, half == 1)
                                asl = patt[:, slot + poff:slot + poff + pw]
                                dsl = dacc[:, poff:poff + pw]
                                if first_chunk:
                                    nc.vector.tensor_copy(dsl, asl)
                                    first_chunk = False
                                else:
                                    nc.vector.tensor_add(dsl, dsl, asl)
                            pend_norm = (o_ps, dacc, h, i)
                        if i > 0 and "C" in parts:
                            while ysub < 16:
                                emit_y(i - 1, ysub)
                                ysub += 1
                    flush_norm(pend_norm)
                    if "C" in parts:
                        for sub in range(16):
                            emit_y(NI - 1, sub)

    nc.compile()
    return nc


_EVOD = None


def _perm():
    global _EVOD
    if _EVOD is None:
        _EVOD = np.concatenate([np.arange(0, HD, 2), np.arange(1, HD, 2)])
    return _EVOD


def prepare_inputs(x, wq, wk, wv, wo, q_norm_w, k_norm_w, freqs_cos, freqs_sin):
    """Host-side sharding + layout prep. Returns (in_maps, shared, unit_w)."""
    import ml_dtypes
    bnp = ml_dtypes.bfloat16

    x = np.asarray(x, np.float32)
    wq = np.asarray(wq, np.float32)
    wk = np.asarray(wk, np.float32)
    wv = np.asarray(wv, np.float32)
    wo = np.asarray(wo, np.float32)
    qw = np.asarray(q_norm_w, np.float32)
    kw = np.asarray(k_norm_w, np.float32)
    cos = np.asarray(freqs_cos, np.float32)
    sin = np.asarray(freqs_sin, np.float32)

    perm = _perm()
    shared = bool(np.allclose(qw, kw))
    unit_w = bool(np.allclose(qw, 1.0) and np.allclose(kw, 1.0))

    def freq4(w):
        we, wo_ = w[0::2], w[1::2]
        return np.concatenate(
            [cos * we[None, :], sin * wo_[None, :],
             sin * we[None, :], cos * wo_[None, :]], axis=1)

    f8 = freq4(qw) if shared else np.concatenate([freq4(qw), freq4(kw)], axis=1)
    f8 = np.ascontiguousarray(f8).astype(bnp)

    msk = np.where(np.arange(128)[None, :] >= np.arange(128)[:, None],
                   np.float32(0.0), np.float32(NEG)).astype(bnp)
    eye = np.eye(128, dtype=np.float32).astype(bnp)
    onc = np.ones((128, 1), np.float16)
    onr = np.ones((1, 128), np.float32)

    xTs = [np.ascontiguousarray(x[b].T).astype(bnp) for b in range(B)]
    in_maps = []
    for c in range(NCORES):
        b, g = divmod(c, KV)
        wq_g = wq[:, g * GW:(g + 1) * GW].reshape(D, NH, HD)[:, :, perm] \
            .reshape(D, GW)
        wk_g = wk[:, g * HD:(g + 1) * HD][:, perm]
        wv_g = wv[:, g * HD:(g + 1) * HD]
        wkv_g = np.ascontiguousarray(
            np.concatenate([wk_g, wv_g], axis=1)).astype(bnp)
        wo_g = np.ascontiguousarray(wo[g * GW:(g + 1) * GW, :]).astype(bnp)
        in_maps.append(dict(
            xT=xTs[b], wq=np.ascontiguousarray(wq_g).astype(bnp),
            wkv=wkv_g, wo=wo_g, f8=f8, msk=msk, eye=eye, onc=onc, onr=onr))
    return in_maps, shared, unit_w


def get_program(shared_freqs: bool, repeat: int = 1, timing: bool = False,
                unit_w: bool = True, parts: str = "ABC"):
    key = (shared_freqs, repeat, timing, unit_w, parts)
    if key not in _prog_cache:
        _prog_cache[key] = _build(shared_freqs, repeat, timing, unit_w, parts)
    return _prog_cache[key]


def kernel(**inputs):
    from concourse.bass_utils import run_bass_kernel_spmd

    in_maps, shared, unit_w = prepare_inputs(**inputs)
    nc = get_program(shared, unit_w=unit_w)
    res = run_bass_kernel_spmd(nc, in_maps, list(range(NCORES)))
    out = np.empty((B, T, D), np.float32)
    for b in range(B):
        acc = res.results[b * KV + 0]["y"].astype(np.float32)
        for g in range(1, KV):
            acc = acc + res.results[b * KV + g]["y"].astype(np.float32)
        out[b] = acc
    return out
